# revision 1
# baseline (speedup 1.0000x reference)
"""Trainium2 Bass kernel for nn_APSDG_30124900614677 (gnn_message_passing).

Sharded over 8 NeuronCores: nodes split contiguously (6250/core, padded to
6272=49*128). Per GNN layer: node-local transforms (e: GraphConv norm,
b: Poincare logmap, s: sphere l2norm) as PE matmuls -> bf16 AllGather of
transformed features -> per-core edge phase over the core's own-dst edges:
dma_gather of src rows from the gathered bf16 table + segment-sum via
selection-matrix matmuls accumulated in PSUM (edges pre-sorted by dst on
host; selection matrices built on-device with one batched is_equal per
group) -> epilogue nonlinearities. History attention + GRU are node-local;
host-folded weight products collapse the k/v projections into one matmul
each (softmax shift-invariance drops the k bias exactly).

Self-contained: hardcodes shapes; uses numpy/ml_dtypes + the concourse Bass
stack staged at /opt/trn_rl_repo.
"""
import os
import sys
import time

sys.path.insert(0, "/opt/trn_rl_repo")

import numpy as np

from concourse import bass, bacc, mybir, tile
from concourse.bass_utils import run_bass_kernel_spmd

try:
    import ml_dtypes
    BF16 = np.dtype(ml_dtypes.bfloat16)
except Exception:  # pragma: no cover
    import jax.numpy as jnp
    BF16 = np.dtype(jnp.bfloat16)

F32 = mybir.dt.float32
BF = mybir.dt.bfloat16
I16 = mybir.dt.int16
AF = mybir.ActivationFunctionType
OP = mybir.AluOpType

# problem constants
N = 50000
E = 800000
D = 384
SUB = 128
W = 5
L = 2
NCORES = 8
OWN = N // NCORES            # 6250
T = 49                       # node tiles per core
NP = T * 128                 # 6272 padded nodes per core
NPAD = NCORES * NP           # 50176
SPLIT = 32768                # int16 gather-table split row
LEAKY = 0.2
EPS = 1e-5
SEPS = 1e-12
KC = 8                       # dma_gather chunk (64+1 descriptors max)

_RUN_STATE = {}


def _pad_rows(a, rows):
    out = np.zeros((rows,) + a.shape[1:], a.dtype)
    out[: a.shape[0]] = a
    return out


# ---------------------------------------------------------------- host prep

def _prep(inputs):
    src = np.asarray(inputs["src"]).astype(np.int64)
    dst = np.asarray(inputs["dst"]).astype(np.int64)
    node_emb = np.asarray(inputs["node_emb"], dtype=np.float32)
    history = np.asarray(inputs["history"], dtype=np.float32)

    deg_out = np.maximum(np.bincount(src, minlength=N), 1.0).astype(np.float32)
    deg_in = np.maximum(np.bincount(dst, minlength=N), 1.0).astype(np.float32)
    isqo = deg_out ** -0.5
    isqi = deg_in ** -0.5
    iin = (1.0 / deg_in).astype(np.float32)

    c_abs = float(np.abs(np.asarray(inputs["curv_b"]).reshape(-1)[0]))

    src_pad = (src // OWN) * NP + (src % OWN)   # global padded src id

    # per-core own-dst edges, grouped by (dst tile, src class)
    core_of = dst // OWN
    percore = []
    cnt = np.zeros((NCORES, T, 2), np.int64)
    for c in range(NCORES):
        m = core_of == c
        sp = src_pad[m]
        dl = dst[m] - c * OWN
        g = dl // 128
        cls = (sp >= SPLIT).astype(np.int64)
        key = g * 2 + cls
        # sort by src within each (group, class) block: gather locality
        order = np.lexsort((sp, key))
        sp, dl, g, cls, key = (a[order] for a in (sp, dl, g, cls, key))
        cnt[c] += np.bincount(key, minlength=T * 2).reshape(T, 2)
        percore.append((sp, dl, g, cls, key))

    KA = np.maximum(1, -(-cnt[:, :, 0].max(0) // 128)).astype(np.int64)
    KB = np.maximum(1, -(-cnt[:, :, 1].max(0) // 128)).astype(np.int64)
    KG = KA + KB
    TILES_TOT = int(KG.sum())
    IDXCOLS = int(8 * KG.sum())

    tile_base = np.zeros((T, 2), np.int64)
    col_base = np.zeros((T, 2), np.int64)
    tb = cb = 0
    for g in range(T):
        tile_base[g, 0] = tb
        tile_base[g, 1] = tb + KA[g]
        col_base[g, 0] = cb
        col_base[g, 1] = cb + 8 * KA[g]
        tb += KG[g]
        cb += 8 * KG[g]

    # ---- fold attention/GRU weights (f64 accumulate) ----
    f64 = np.float64
    q_W = np.asarray(inputs["q_W"], f64)
    q_b = np.asarray(inputs["q_b"], f64)
    in_w = np.asarray(inputs["in_w"], f64)
    in_b = np.asarray(inputs["in_b"], f64)
    out_w = np.asarray(inputs["out_w"], f64)
    out_b = np.asarray(inputs["out_b"], f64)
    gru_wih = np.asarray(inputs["gru_wih"], f64)
    gru_bih = np.asarray(inputs["gru_bih"], f64)
    gru_whh = np.asarray(inputs["gru_whh"], f64)
    gru_bhh = np.asarray(inputs["gru_bhh"], f64)

    Wq = q_W @ in_w[:, :D]
    aq = q_b @ in_w[:, :D] + in_b[:D]
    Wk = in_w[:, D:2 * D]
    Wqk = Wq @ Wk.T
    bqk = aq @ Wk.T
    Wv = in_w[:, 2 * D:]
    bv = in_b[2 * D:]
    Wvo = Wv @ out_w
    bvo = bv @ out_w + out_b
    Wvog = Wvo @ gru_whh
    bvog = bvo @ gru_whh + gru_bhh

    def kmaj(wmat, n):
        return np.ascontiguousarray(
            wmat.reshape(3, 128, n).transpose(1, 0, 2)).astype(BF16)

    wqk_h = kmaj(Wqk, D)
    wvo_h = kmaj(Wvo, D)
    wvog_h = kmaj(Wvog, 3 * D)
    wih_h = kmaj(gru_wih, 3 * D)

    battn = np.zeros((4, 3 * D), np.float64)
    battn[0, :D] = bqk
    battn[1, :D] = bvo
    battn[2] = bvog
    battn[3] = gru_bih
    battn_h = battn.astype(BF16)
    battn_nz = [bool(np.abs(battn[i]).max() > 0) for i in range(4)]

    e_W = np.asarray(inputs["e_W"], np.float32)
    b_W = np.asarray(inputs["b_W"], np.float32)
    s_W = np.asarray(inputs["s_W"], np.float32)
    wsub_h = np.zeros((128, 2 * 3, 128), BF16)
    for l in range(L):
        wsub_h[:, l * 3 + 0] = e_W[l].astype(BF16)
        wsub_h[:, l * 3 + 1] = b_W[l].astype(BF16)
        wsub_h[:, l * 3 + 2] = s_W[l].astype(BF16)

    e_b = np.asarray(inputs["e_b"], np.float32)
    b_bias = np.asarray(inputs["b_bias"], np.float32)
    s_bias = np.asarray(inputs["s_bias"], np.float32)
    brep_h = np.zeros((128, 6, 128), np.float32)
    brep_nz = np.zeros((L, 3), bool)
    for l in range(L):
        for i, b in enumerate((e_b[l], b_bias[l], s_bias[l])):
            brep_h[:, l * 3 + i] = b[None, :]
            brep_nz[l, i] = bool(np.abs(b).max() > 0)

    iota_h = np.tile(np.arange(128, dtype=np.float32)[None, :], (128, 1)).astype(BF16)
    idf_h = np.eye(128, dtype=np.float32)
    idb_h = np.eye(128, dtype=np.float32).astype(BF16)

    meta = dict(KA=KA, KB=KB, KG=KG, TILES_TOT=TILES_TOT, IDXCOLS=IDXCOLS,
                tile_base=tile_base, col_base=col_base, c_abs=c_abs,
                battn_nz=battn_nz, brep_nz=brep_nz)

    in_maps = []
    for c in range(NCORES):
        sp, dl, g, cls, key = percore[c]
        idx16 = np.zeros((16, IDXCOLS), np.int16)
        drel = np.full((128, TILES_TOT), -1.0, BF16)

        starts = np.zeros(T * 2, np.int64)
        bc = np.bincount(key, minlength=T * 2)
        starts[1:] = np.cumsum(bc)[:-1]
        rank = np.arange(len(sp)) - starts[key]

        val = np.where(cls == 0, sp, sp - SPLIT).astype(np.int16)
        col = col_base[g, cls] + rank // 16
        idx16[rank % 16, col] = val

        tt = tile_base[g, cls] + rank // 128
        drel[rank % 128, tt] = (dl - g * 128).astype(BF16)

        idx_h = np.tile(idx16, (8, 1))

        lo = c * OWN
        hi = lo + OWN
        scales = np.ones((128, 3 * T), np.float32)
        for j, arr in enumerate((isqo, isqi, iin)):
            own = _pad_rows(arr[lo:hi, None], NP)[:, 0]
            scales[:, j * T:(j + 1) * T] = own.reshape(T, 128).T

        feat0 = _pad_rows(node_emb[lo:hi], NP).astype(BF16)
        hist = np.zeros((NP, W * D), BF16)
        hist[:OWN] = np.ascontiguousarray(
            history[:, lo:hi, :].transpose(1, 0, 2)
        ).reshape(OWN, W * D).astype(BF16)

        in_maps.append({
            "feat0": feat0,
            "hist": hist,
            "idx": idx_h,
            "drel": drel,
            "scales": scales,
            "wsub": wsub_h.reshape(128, 6 * 128),
            "brep": brep_h.reshape(128, 6 * 128),
            "wqk": wqk_h.reshape(128, 3 * D),
            "wvo": wvo_h.reshape(128, 3 * D),
            "wvog": wvog_h.reshape(128, 9 * D),
            "wih": wih_h.reshape(128, 9 * D),
            "battn": battn_h,
            "iota": iota_h,
            "idf": idf_h,
            "idb": idb_h,
        })
    return in_maps, meta


# ------------------------------------------------------------- device build

def _build(meta):
    KA, KB, KG = meta["KA"], meta["KB"], meta["KG"]
    TILES_TOT, IDXCOLS = meta["TILES_TOT"], meta["IDXCOLS"]
    tile_base, col_base = meta["tile_base"], meta["col_base"]
    c_abs = meta["c_abs"]
    battn_nz = meta["battn_nz"]
    brep_nz = meta["brep_nz"]
    ID_SCALE = float(D) ** -0.5
    phases = os.environ.get("GNN_PHASES", "taex")
    single = bool(os.environ.get("GNN_SINGLE"))  # 1-core cost-model variant
    if single:
        phases = phases.replace("a", "")

    nc = bacc.Bacc("TRN2", target_bir_lowering=False, debug=False,
                   num_devices=1 if single else NCORES)

    feat0_d = nc.dram_tensor("feat0", [NP, D], BF, kind="ExternalInput")
    hist_d = nc.dram_tensor("hist", [NP, W * D], BF, kind="ExternalInput")
    idx_d = nc.dram_tensor("idx", [128, IDXCOLS], I16, kind="ExternalInput")
    drel_d = nc.dram_tensor("drel", [128, TILES_TOT], BF, kind="ExternalInput")
    scales_d = nc.dram_tensor("scales", [128, 3 * T], F32, kind="ExternalInput")
    wsub_d = nc.dram_tensor("wsub", [128, 6 * 128], BF, kind="ExternalInput")
    brep_d = nc.dram_tensor("brep", [128, 6 * 128], F32, kind="ExternalInput")
    wqk_d = nc.dram_tensor("wqk", [128, 3 * D], BF, kind="ExternalInput")
    wvo_d = nc.dram_tensor("wvo", [128, 3 * D], BF, kind="ExternalInput")
    wvog_d = nc.dram_tensor("wvog", [128, 9 * D], BF, kind="ExternalInput")
    wih_d = nc.dram_tensor("wih", [128, 9 * D], BF, kind="ExternalInput")
    battn_d = nc.dram_tensor("battn", [4, 3 * D], BF, kind="ExternalInput")
    iota_d = nc.dram_tensor("iota", [128, 128], BF, kind="ExternalInput")
    idf_d = nc.dram_tensor("idf", [128, 128], F32, kind="ExternalInput")
    idb_d = nc.dram_tensor("idb", [128, 128], BF, kind="ExternalInput")
    out_d = nc.dram_tensor("out", [NP, D], F32, kind="ExternalOutput")

    t_own = nc.dram_tensor("t_own", [NP, D], BF, kind="Internal")
    t_full = nc.dram_tensor("t_full", [NPAD, D], BF, kind="Internal",
                            addr_space="Shared")
    NB = 7  # batch width: tiles/groups per chain batch (49 = 7x7)

    with tile.TileContext(nc) as tc:
        with tc.tile_pool(name="const", bufs=1) as cpool:
            idx_sb = cpool.tile([128, IDXCOLS], I16)
            drel_sb = cpool.tile([128, TILES_TOT], BF)
            scales_sb = cpool.tile([128, 3 * T], F32)
            wsub_sb = cpool.tile([128, 6 * 128], BF)
            brep_sb = cpool.tile([128, 6 * 128], F32)
            wqk_sb = cpool.tile([128, 3 * D], BF)
            wvo_sb = cpool.tile([128, 3 * D], BF)
            wvog_sb = cpool.tile([128, 9 * D], BF)
            wih_sb = cpool.tile([128, 9 * D], BF)
            battn_sb = cpool.tile([4, 3 * D], BF)
            iota_sb = cpool.tile([128, 128], BF)
            idf_sb = cpool.tile([128, 128], F32)
            idb_sb = cpool.tile([128, 128], BF)
            ones_sb = cpool.tile([1, 128], BF)
            for sb, dr in ((idx_sb, idx_d), (drel_sb, drel_d),
                           (scales_sb, scales_d), (wsub_sb, wsub_d),
                           (brep_sb, brep_d), (wqk_sb, wqk_d),
                           (wvo_sb, wvo_d), (wvog_sb, wvog_d),
                           (wih_sb, wih_d), (battn_sb, battn_d),
                           (iota_sb, iota_d), (idf_sb, idf_d),
                           (idb_sb, idb_d)):
                nc.sync.dma_start(sb[:], dr[:])
            nc.gpsimd.memset(ones_sb[:], 1.0)

            V = nc.vector
            S_ = nc.scalar

            def chains_batch(pool, ssb, sss, nb, kind_b):
                """Batched norm-scale chains on [128, nb] sumsq columns.

                ssb -> b-subspace scale ('log': artanh(min(sn,1-e))/max(sn,e);
                'exp': tanh(sn)/max(sn,e)); sss -> l2 scale
                1/max(sqrt(sss),1e-12). Sqrts issued back-to-back to bound
                ACT-table switches to <=3 per batch."""
                snb = pool.tile([128, NB], F32, tag="c_snb")
                S_.activation(snb[:, :nb], ssb[:, :nb], AF.Sqrt, scale=c_abs)
                sns = pool.tile([128, NB], F32, tag="c_sns")
                S_.activation(sns[:, :nb], sss[:, :nb], AF.Sqrt)
                msc = pool.tile([128, NB], F32, tag="c_msc")
                V.tensor_scalar_max(msc[:, :nb], sns[:, :nb], SEPS)
                V.reciprocal(msc[:, :nb], msc[:, :nb])
                m = pool.tile([128, NB], F32, tag="c_m")
                V.tensor_scalar_max(m[:, :nb], snb[:, :nb], EPS)
                V.reciprocal(m[:, :nb], m[:, :nb])
                bsc = pool.tile([128, NB], F32, tag="c_bsc")
                if kind_b == "exp":
                    th = pool.tile([128, NB], F32, tag="c_th")
                    S_.activation(th[:, :nb], snb[:, :nb], AF.Tanh)
                    V.tensor_tensor(out=bsc[:, :nb], in0=th[:, :nb],
                                    in1=m[:, :nb], op=OP.mult)
                else:
                    x = pool.tile([128, NB], F32, tag="c_x")
                    V.tensor_scalar_min(x[:, :nb], snb[:, :nb], 1.0 - EPS)
                    a1 = pool.tile([128, NB], F32, tag="c_a1")
                    V.tensor_scalar_add(a1[:, :nb], x[:, :nb], 1.0)
                    a2 = pool.tile([128, NB], F32, tag="c_a2")
                    V.tensor_scalar(a2[:, :nb], x[:, :nb], -1.0, 1.0,
                                    op0=OP.mult, op1=OP.add)
                    V.reciprocal(a2[:, :nb], a2[:, :nb])
                    y = pool.tile([128, NB], F32, tag="c_y")
                    V.tensor_tensor(out=y[:, :nb], in0=a1[:, :nb],
                                    in1=a2[:, :nb], op=OP.mult)
                    ln = pool.tile([128, NB], F32, tag="c_ln")
                    S_.activation(ln[:, :nb], y[:, :nb], AF.Ln)
                    V.scalar_tensor_tensor(out=bsc[:, :nb], in0=ln[:, :nb],
                                           scalar=0.5, in1=m[:, :nb],
                                           op0=OP.mult, op1=OP.mult)
                return bsc, msc

            def transform_batch(l, b0, nb, ft_b, pool, pp):
                """One batch of the node-local transform: features
                ft_b [128, nb, D] (bf16) -> t_own rows [b0*128, (b0+nb)*128)."""
                ssb = pool.tile([128, NB], F32, tag="t_ssb")
                sss = pool.tile([128, NB], F32, tag="t_sss")
                junk = pool.tile([128, 128], BF, tag="t_junk")
                for j in range(nb):
                    S_.activation(junk[:], ft_b[:, j, 128:256], AF.Square,
                                  accum_out=ssb[:, j:j + 1])
                    S_.activation(junk[:], ft_b[:, j, 256:384], AF.Square,
                                  accum_out=sss[:, j:j + 1])
                bsc, msc = chains_batch(pool, ssb, sss, nb, "log")
                tsbb = pool.tile([128, NB, D], BF, tag="t_tsbb")
                for j in range(nb):
                    t = b0 + j
                    ft = ft_b[:, j, :]
                    tsb = tsbb[:, j, :]
                    for i, kind in enumerate(("e", "b", "s")):
                        sl = slice(i * 128, (i + 1) * 128)
                        if kind == "e":
                            xin = ft[:, sl]
                        else:
                            sc = bsc if kind == "b" else msc
                            xs = pool.tile([128, 128], BF, tag="t_xs")
                            V.tensor_scalar_mul(xs[:], ft[:, sl], sc[:, j:j + 1])
                            xin = xs[:]
                        pt = pp.tile([128, 128], BF, tag="tp")
                        nc.tensor.transpose(pt[:], xin, idb_sb[:])
                        xT = pool.tile([128, 128], BF, tag="t_xT")
                        V.tensor_copy(xT[:], pt[:])
                        pm = pp.tile([128, 128], F32, tag="mm")
                        nc.tensor.matmul(
                            pm[:], lhsT=xT[:],
                            rhs=wsub_sb[:, (l * 3 + i) * 128:(l * 3 + i + 1) * 128],
                            start=True, stop=True)
                        if kind == "e":
                            V.tensor_scalar_mul(tsb[:, sl], pm[:],
                                                scales_sb[:, t:t + 1])
                        elif brep_nz[l][i]:
                            V.scalar_tensor_tensor(
                                out=tsb[:, sl], in0=pm[:], scalar=1.0,
                                in1=brep_sb[:, (l * 3 + i) * 128:(l * 3 + i + 1) * 128],
                                op0=OP.mult, op1=OP.add)
                        else:
                            V.tensor_copy(tsb[:, sl], pm[:])
                nc.sync.dma_start(
                    t_own[b0 * 128:(b0 + nb) * 128, :].rearrange(
                        "(b p) d -> p b d", p=128),
                    tsbb[:, :nb, :])

            def edge_batch(l, b0, nb, pool, pp):
                """Edge propagation for groups [b0, b0+nb): returns the
                finished feature batch tile stage [128, nb, D] (bf16)."""
                KGmax = int(KG.max())
                ssb = pool.tile([128, NB], F32, tag="e_ssb")
                sss = pool.tile([128, NB], F32, tag="e_sss")
                junk = pool.tile([128, 128], BF, tag="e_junk")
                stage = pool.tile([128, NB, D], BF, tag="e_stage")
                for j in range(nb):
                    g = b0 + j
                    ka, kb, kg = int(KA[g]), int(KB[g]), int(KG[g])
                    ca, cbb = int(col_base[g, 0]), int(col_base[g, 1])
                    tb_ = int(tile_base[g, 0])
                    msg = pool.tile([128, KGmax, D], BF, tag="e_msg")

                    def gather(table_ap, kcnt, colofs, chunk0):
                        for q0 in range(0, kcnt, KC):
                            kc = min(KC, kcnt - q0)
                            nc.gpsimd.dma_gather(
                                out_ap=msg[:, chunk0 + q0:chunk0 + q0 + kc, :],
                                in_ap=table_ap,
                                idxs_ap=idx_sb[:, colofs + 8 * q0:
                                               colofs + 8 * (q0 + kc)],
                                num_idxs=128 * kc, num_idxs_reg=128 * kc,
                                elem_size=D)

                    gather(t_full[0:SPLIT, :], ka, ca, 0)
                    gather(t_full[SPLIT:NPAD, :], kb, cbb, ka)

                    Sall = pool.tile([128, KGmax * 128], BF, tag="e_S")
                    V.tensor_tensor(
                        out=Sall[:].rearrange("p (k j) -> p k j", j=128)[:, :kg, :],
                        in0=drel_sb[:, tb_:tb_ + kg].to_broadcast([128, kg, 128]),
                        in1=iota_sb[:, None, :].to_broadcast([128, kg, 128]),
                        op=OP.is_equal)
                    ps = pp.tile([128, D], F32, tag="eps")
                    for k in range(kg):
                        nc.tensor.matmul(ps[:],
                                         lhsT=Sall[:, k * 128:(k + 1) * 128],
                                         rhs=msg[:, k, :],
                                         start=(k == 0), stop=(k == kg - 1))
                    # e: *isqi (+e_b) then leaky -> stage. Prelu (in every
                    # ACT table set) fuses the per-dst scale and the leak.
                    if brep_nz[l][0]:
                        et = pool.tile([128, 128], F32, tag="e_et")
                        V.scalar_tensor_tensor(
                            out=et[:], in0=ps[:, 0:128],
                            scalar=scales_sb[:, T + g:T + g + 1],
                            in1=brep_sb[:, (l * 3) * 128:(l * 3 + 1) * 128],
                            op0=OP.mult, op1=OP.add)
                        S_.activation(stage[:, j, 0:128], et[:], AF.Prelu,
                                      alpha=LEAKY)
                    else:
                        S_.activation(stage[:, j, 0:128], ps[:, 0:128],
                                      AF.Prelu,
                                      scale=scales_sb[:, T + g:T + g + 1],
                                      alpha=LEAKY)
                    # b: u = ps*iin -> stage (pre-scale), sumsq col
                    S_.activation(stage[:, j, 128:256], ps[:, 128:256], AF.Copy,
                                  scale=scales_sb[:, 2 * T + g:2 * T + g + 1])
                    S_.activation(junk[:], stage[:, j, 128:256], AF.Square,
                                  accum_out=ssb[:, j:j + 1])
                    # s: raw copy -> stage, sumsq col (inv_in cancels in l2)
                    S_.activation(junk[:], ps[:, 256:384], AF.Square,
                                  accum_out=sss[:, j:j + 1])
                    S_.copy(stage[:, j, 256:384], ps[:, 256:384])
                bsc, msc = chains_batch(pool, ssb, sss, nb, "exp")
                for j in range(nb):
                    V.tensor_scalar_mul(stage[:, j, 128:256],
                                        stage[:, j, 128:256], bsc[:, j:j + 1])
                    V.tensor_scalar_mul(stage[:, j, 256:384],
                                        stage[:, j, 256:384], msc[:, j:j + 1])
                return stage

            def attention_batch(b0, nb, stage, pool, hpool, pp1, ppq, pp2):
                hbb = hpool.tile([128, NB, W * D], BF, tag="hbb")
                nc.sync.dma_start(
                    hbb[:, :nb, :],
                    hist_d[b0 * 128:(b0 + nb) * 128, :].rearrange(
                        "(b p) d -> p b d", p=128))
                otb = pool.tile([128, NB, D], F32, tag="otb")
                for j in range(nb):
                    cur = stage[:, j, :]
                    hb = hbb[:, j, :]
                    curT = []
                    for i in range(3):
                        pt = pp2.tile([128, 128], BF, tag="tp")
                        nc.tensor.transpose(pt[:], cur[:, i * 128:(i + 1) * 128],
                                            idb_sb[:])
                        cT = pool.tile([128, 128], BF, tag=f"cT{i}")
                        S_.copy(cT[:], pt[:])
                        curT.append(cT)

                    ps_q = ppq.tile([128, D], F32, tag="pqc")
                    ps_r = pp1.tile([128, D], F32, tag="pr")
                    ps_z = pp1.tile([128, D], F32, tag="pz")
                    ps_n1 = pp1.tile([128, D], F32, tag="pn1")
                    ps_n2 = pp1.tile([128, D], F32, tag="pn2")

                    for i in range(3):
                        nc.tensor.matmul(ps_q[:], lhsT=curT[i][:],
                                         rhs=wqk_sb[:, i * D:(i + 1) * D],
                                         start=(i == 0),
                                         stop=(i == 2 and not battn_nz[0]))
                    if battn_nz[0]:
                        nc.tensor.matmul(ps_q[:], lhsT=ones_sb[:],
                                         rhs=battn_sb[0:1, 0:D],
                                         start=False, stop=True)
                    for i in range(3):
                        nc.tensor.matmul(ps_r[:], lhsT=curT[i][:],
                                         rhs=wih_sb[:, i * 3 * D:i * 3 * D + D],
                                         start=(i == 0), stop=False)
                        nc.tensor.matmul(ps_z[:], lhsT=curT[i][:],
                                         rhs=wih_sb[:, i * 3 * D + D:i * 3 * D + 2 * D],
                                         start=(i == 0), stop=False)
                        nc.tensor.matmul(ps_n1[:], lhsT=curT[i][:],
                                         rhs=wih_sb[:, i * 3 * D + 2 * D:(i + 1) * 3 * D],
                                         start=(i == 0),
                                         stop=(i == 2 and not battn_nz[3]))
                    if battn_nz[3]:
                        nc.tensor.matmul(ps_r[:], lhsT=ones_sb[:],
                                         rhs=battn_sb[3:4, 0:D], start=False,
                                         stop=False)
                        nc.tensor.matmul(ps_z[:], lhsT=ones_sb[:],
                                         rhs=battn_sb[3:4, D:2 * D], start=False,
                                         stop=False)
                        nc.tensor.matmul(ps_n1[:], lhsT=ones_sb[:],
                                         rhs=battn_sb[3:4, 2 * D:3 * D],
                                         start=False, stop=True)

                    qs = pool.tile([128, D], BF, tag="qs")
                    S_.activation(qs[:], ps_q[:], AF.Copy)

                    sc_t = pool.tile([128, W], F32, tag="sc")
                    junkb = pool.tile([128, D], BF, tag="junkb")
                    for w in range(W):
                        V.scalar_tensor_tensor(
                            out=junkb[:], in0=qs[:], scalar=1.0,
                            in1=hb[:, w * D:(w + 1) * D],
                            op0=OP.mult, op1=OP.mult,
                            accum_out=sc_t[:, w:w + 1])
                    mx = pool.tile([128, 1], F32, tag="mx")
                    V.reduce_max(mx[:], sc_t[:], axis=mybir.AxisListType.X)
                    nmx = pool.tile([128, 1], F32, tag="nmx")
                    V.tensor_scalar_mul(nmx[:], mx[:], -ID_SCALE)
                    ex = pool.tile([128, W], F32, tag="ex")
                    den = pool.tile([128, 1], F32, tag="den")
                    S_.activation(ex[:], sc_t[:], AF.Exp, bias=nmx[:, 0:1],
                                  scale=ID_SCALE, accum_out=den[:])
                    V.reciprocal(den[:], den[:])
                    at_t = pool.tile([128, W], F32, tag="at")
                    V.tensor_scalar_mul(at_t[:], ex[:], den[:, 0:1])

                    acc = pool.tile([128, D], F32, tag="acc")
                    acc2 = pool.tile([128, D], F32, tag="acc2")
                    V.tensor_scalar_mul(acc[:], hb[:, 0:D], at_t[:, 0:1])
                    for w in range(1, W):
                        a_in, a_out = (acc, acc2) if w % 2 == 1 else (acc2, acc)
                        V.scalar_tensor_tensor(
                            out=a_out[:], in0=hb[:, w * D:(w + 1) * D],
                            scalar=at_t[:, w:w + 1], in1=a_in[:],
                            op0=OP.mult, op1=OP.add)
                    ctx_pre = acc if (W - 1) % 2 == 0 else acc2

                    cpT = []
                    for i in range(3):
                        pt = pp2.tile([128, 128], F32, tag="tp")
                        nc.tensor.transpose(pt[:], ctx_pre[:, i * 128:(i + 1) * 128],
                                            idf_sb[:])
                        cT = pool.tile([128, 128], BF, tag=f"vT{i}")
                        S_.copy(cT[:], pt[:])
                        cpT.append(cT)

                    ps_c = ppq.tile([128, D], F32, tag="pqc")
                    for i in range(3):
                        nc.tensor.matmul(ps_c[:], lhsT=cpT[i][:],
                                         rhs=wvo_sb[:, i * D:(i + 1) * D],
                                         start=(i == 0),
                                         stop=(i == 2 and not battn_nz[1]))
                        nc.tensor.matmul(ps_r[:], lhsT=cpT[i][:],
                                         rhs=wvog_sb[:, i * 3 * D:i * 3 * D + D],
                                         start=False,
                                         stop=(i == 2 and not battn_nz[2]))
                        nc.tensor.matmul(ps_z[:], lhsT=cpT[i][:],
                                         rhs=wvog_sb[:, i * 3 * D + D:i * 3 * D + 2 * D],
                                         start=False,
                                         stop=(i == 2 and not battn_nz[2]))
                        nc.tensor.matmul(ps_n2[:], lhsT=cpT[i][:],
                                         rhs=wvog_sb[:, i * 3 * D + 2 * D:(i + 1) * 3 * D],
                                         start=(i == 0),
                                         stop=(i == 2 and not battn_nz[2]))
                    if battn_nz[1]:
                        nc.tensor.matmul(ps_c[:], lhsT=ones_sb[:],
                                         rhs=battn_sb[1:2, 0:D], start=False,
                                         stop=True)
                    if battn_nz[2]:
                        nc.tensor.matmul(ps_r[:], lhsT=ones_sb[:],
                                         rhs=battn_sb[2:3, 0:D], start=False,
                                         stop=True)
                        nc.tensor.matmul(ps_z[:], lhsT=ones_sb[:],
                                         rhs=battn_sb[2:3, D:2 * D], start=False,
                                         stop=True)
                        nc.tensor.matmul(ps_n2[:], lhsT=ones_sb[:],
                                         rhs=battn_sb[2:3, 2 * D:3 * D],
                                         start=False, stop=True)

                    # sigmoid(x) = 0.5*(tanh(x/2)+1): stays in the exp/tanh
                    # ACT table set (no reloads).
                    ctx = pool.tile([128, D], F32, tag="ctx")
                    S_.activation(ctx[:], ps_c[:], AF.Copy)
                    tr = pool.tile([128, D], F32, tag="rs")
                    S_.activation(tr[:], ps_r[:], AF.Tanh, scale=0.5)
                    tz = pool.tile([128, D], F32, tag="zs")
                    S_.activation(tz[:], ps_z[:], AF.Tanh, scale=0.5)
                    ar = pool.tile([128, D], F32, tag="ar")
                    V.tensor_scalar_add(ar[:], tr[:], 1.0)
                    prod = pool.tile([128, D], F32, tag="prod")
                    V.scalar_tensor_tensor(out=prod[:], in0=ar[:], scalar=0.5,
                                           in1=ps_n2[:], op0=OP.mult, op1=OP.mult)
                    pre = pool.tile([128, D], F32, tag="pre")
                    V.tensor_tensor(out=pre[:], in0=prod[:], in1=ps_n1[:],
                                    op=OP.add)
                    nm = pool.tile([128, D], F32, tag="nm")
                    S_.activation(nm[:], pre[:], AF.Tanh)
                    dd = pool.tile([128, D], F32, tag="dd")
                    V.tensor_tensor(out=dd[:], in0=ctx[:], in1=nm[:],
                                    op=OP.subtract)
                    az = pool.tile([128, D], F32, tag="az")
                    V.tensor_scalar_add(az[:], tz[:], 1.0)
                    ot = pool.tile([128, D], F32, tag="ot")
                    V.scalar_tensor_tensor(out=ot[:], in0=az[:], scalar=0.5,
                                           in1=dd[:], op0=OP.mult, op1=OP.mult)
                    V.tensor_tensor(out=otb[:, j, :], in0=ot[:], in1=nm[:],
                                    op=OP.add)
                nc.sync.dma_start(
                    out_d[b0 * 128:(b0 + nb) * 128, :].rearrange(
                        "(b p) d -> p b d", p=128),
                    otb[:, :nb, :])

            # ================= schedule =================
            # transform-1 (own nodes, from feat0)
            if "t" in phases:
                with (
                    tc.tile_pool(name="tf0", bufs=2) as pool,
                    tc.tile_pool(name="tfp0", bufs=2, space="PSUM") as pp,
                ):
                    for b0 in range(0, T, NB):
                        nb = min(NB, T - b0)
                        ftb = pool.tile([128, NB, D], BF, tag="t_ftb")
                        nc.sync.dma_start(
                            ftb[:, :nb, :],
                            feat0_d[b0 * 128:(b0 + nb) * 128, :].rearrange(
                                "(b p) d -> p b d", p=128))
                        transform_batch(0, b0, nb, ftb[:, :nb, :], pool, pp)

            if "a" in phases:
                nc.gpsimd.collective_compute(
                    "AllGather", OP.bypass,
                    replica_groups=[list(range(NCORES))],
                    ins=[t_own[:].opt()], outs=[t_full[:].opt()])

            # edge-1 fused with transform-2
            if "e" in phases:
                with (
                    tc.tile_pool(name="f1", bufs=2) as pool,
                    tc.tile_pool(name="f1p", bufs=2, space="PSUM") as pp,
                ):
                    for b0 in range(0, T, NB):
                        nb = min(NB, T - b0)
                        stage = edge_batch(0, b0, nb, pool, pp)
                        if "t" in phases:
                            transform_batch(1, b0, nb, stage[:, :nb, :],
                                            pool, pp)

                if "a" in phases:
                    nc.gpsimd.collective_compute(
                        "AllGather", OP.bypass,
                        replica_groups=[list(range(NCORES))],
                        ins=[t_own[:].opt()], outs=[t_full[:].opt()])

            # edge-2 fused with attention + GRU
            if "x" in phases:
                with (
                    tc.tile_pool(name="f2", bufs=2) as pool,
                    tc.tile_pool(name="f2h", bufs=1) as hpool,
                    tc.tile_pool(name="f2pe", bufs=1, space="PSUM") as ppE,
                    tc.tile_pool(name="f2pa", bufs=1, space="PSUM") as ppA,
                    tc.tile_pool(name="f2pq", bufs=2, space="PSUM") as ppQ,
                    tc.tile_pool(name="f2pt", bufs=1, space="PSUM") as ppT,
                ):
                    for b0 in range(0, T, NB):
                        nb = min(NB, T - b0)
                        stage = edge_batch(1, b0, nb, pool, ppE)
                        attention_batch(b0, nb, stage, pool, hpool, ppA, ppQ,
                                        ppT)

    nc.compile()
    if os.environ.get("GNN_VERBOSE"):
        n_inst = sum(len(bb.instructions) for f in nc.m.functions for bb in f.blocks)
        print(f"[kernel] instructions: {n_inst}", file=sys.stderr)
    return nc


# ----------------------------------------------------------------- runners

def kernel(**inputs) -> np.ndarray:
    t0 = time.time()
    in_maps, meta = _prep(inputs)
    t1 = time.time()
    nc = _build(meta)
    t2 = time.time()
    res = run_bass_kernel_spmd(nc, in_maps, core_ids=list(range(NCORES)))
    t3 = time.time()
    if os.environ.get("GNN_VERBOSE"):
        print(f"[kernel] prep {t1-t0:.1f}s build+compile {t2-t1:.1f}s "
              f"run {t3-t2:.1f}s", file=sys.stderr)
    _RUN_STATE["nc"] = nc
    _RUN_STATE["in_maps"] = in_maps
    out = np.concatenate([res.results[c]["out"][:OWN] for c in range(NCORES)], 0)
    return out.astype(np.float32)


def bench_prepare():
    """Build a zero-transfer runner for the last kernel() invocation."""
    import jax
    from jax.sharding import Mesh, PartitionSpec, NamedSharding
    from jax.experimental.shard_map import shard_map
    from concourse import bass2jax
    from concourse.bass2jax import _bass_exec_p, install_neuronx_cc_hook

    nc = _RUN_STATE["nc"]
    in_maps = _RUN_STATE["in_maps"]
    install_neuronx_cc_hook()

    part_name = nc.partition_id_tensor.name if nc.partition_id_tensor else None
    in_names, out_names, out_avals, zero_outs = [], [], [], []
    for alloc in nc.m.functions[0].allocations:
        if not isinstance(alloc, mybir.MemoryLocationSet):
            continue
        name = alloc.memorylocations[0].name
        if alloc.kind == "ExternalInput":
            if name != part_name:
                in_names.append(name)
        elif alloc.kind == "ExternalOutput":
            out_names.append(name)
            shape = tuple(alloc.tensor_shape)
            dtype = mybir.dt.np(alloc.dtype)
            out_avals.append(jax.core.ShapedArray(shape, dtype))
            zero_outs.append(np.zeros(shape, dtype))
    n_params = len(in_names)
    all_names = in_names + out_names
    if part_name is not None:
        all_names = all_names + [part_name]

    def _body(*args):
        operands = list(args)
        if part_name is not None:
            operands.append(bass2jax.partition_id_tensor())
        outs = _bass_exec_p.bind(
            *operands, out_avals=tuple(out_avals), in_names=tuple(all_names),
            out_names=tuple(out_names), lowering_input_output_aliases=(),
            sim_require_finite=True, sim_require_nnan=True, nc=nc)
        return tuple(outs)

    devices = jax.devices()[:NCORES]
    mesh = Mesh(np.asarray(devices), ("core",))
    nin = n_params + len(zero_outs)
    fn = jax.jit(shard_map(_body, mesh=mesh,
                           in_specs=(PartitionSpec("core"),) * nin,
                           out_specs=(PartitionSpec("core"),) * len(out_names),
                           check_rep=False))
    sh = NamedSharding(mesh, PartitionSpec("core"))
    concat_in = [
        jax.device_put(np.concatenate([in_maps[c][k] for c in range(NCORES)], 0), sh)
        for k in in_names
    ] + [
        jax.device_put(np.zeros((NCORES * z.shape[0], *z.shape[1:]), z.dtype), sh)
        for z in zero_outs
    ]

    def run_once():
        out = fn(*concat_in)
        jax.block_until_ready(out)

    run_once()
    return run_once


def bench(iters: int = 8) -> float:
    run_once = bench_prepare()
    best = float("inf")
    for _ in range(iters):
        t0 = time.perf_counter()
        run_once()
        best = min(best, time.perf_counter() - t0)
    return best

